# revision 9
# baseline (speedup 1.0000x reference)
"""Linear attention (elu+1 feature map) Bass/Tile kernel for Trainium2.

Full inputs: queries/keys/values [N=8, L/S=8192, H=8, D=64] fp32.
Sharding: data-parallel over N across the 8 NeuronCores (batch i -> core i).

Math per (n, h):
  Q' = elu(Q)+1, K' = elu(K)+1
  KV[d, v] = sum_s K'[s, d] V[s, v]     (the /S, *S in the reference cancel
  Ksum[d]  = sum_s K'[s, d]              exactly in fp32: S = 2^13)
  out[l, v] = (Q'[l, :] @ KV[:, v]) / (Q'[l, :] @ Ksum + eps)

Kernel structure per core:
  Phase 1 (stream K, V):  per 128-row chunk, feature-map K on ACT+DVE, then
    per head one matmul  lhsT=K'_h [128, 64], rhs=[V_h | ones] [128, 65]
    accumulated into PSUM [KV | Ksum].  Head pairs share a PSUM bank via
    tile_position col tiling ((0,0) / (0,64)).
  Phase 2 (stream Q): per 128-row chunk, PE-transpose raw Q ([128 l, 128 2d]
    -> [128 2d, 128 l]), apply elu+1 during the PSUM drain (ACT relu(-x),
    ACT exp(-t), DVE (max(x,0)+e)), then block-diag matmul
    lhsT=Q'^T-pair [128, 128], rhs=W2aug [128, 130] -> psum [128 l, 65+65]
    with out columns and the denominator column per head; epilogue divides
    on DVE and DMAs out in natural [l, (h v)] layout.
"""

import functools
import sys

sys.path.insert(0, "/opt/trn_rl_repo")

import numpy as np

import concourse.bass as bass
import concourse.mybir as mybir
import concourse.tile as tile
from concourse import bacc
from concourse.bass_utils import run_bass_kernel_spmd
from concourse.masks import make_identity

N, L, S, H, D = 8, 8192, 8192, 8, 64
EPS = 1e-6
P = 128
FP32 = mybir.dt.float32
AF = mybir.ActivationFunctionType
OP = mybir.AluOpType


def _feature_map(nc, pools, x_ap, out_ap, shape, tag, split=False):
    """out = elu(x)+1 = max(x,0) + exp(min(x,0)).

    Fused form (split=False): ACT t = relu(-x); ACT e = exp(-t);
    DVE out = (x max 0) + e.  Used when x comes from PSUM (PE) so the DVE
    op sees only 2 distinct upstream semaphores (PE + ACT).

    Split form (split=True): same t, e; then DVE s = t + e;
    DVE out = x + s  (relu(x) = x + relu(-x), so x + t + e = elu(x)+1).
    Keeps every instruction at <=2 distinct semaphore waits when x comes
    from a DMA (walrus rejects >2 sync waits per ACT/STT instruction).
    """
    t = pools.tile(shape, FP32, name=f"fm_t_{tag}", tag=f"fm_t_{tag}")
    e = pools.tile(shape, FP32, name=f"fm_e_{tag}", tag=f"fm_e_{tag}")
    nc.scalar.activation(t, x_ap, AF.Relu, scale=-1.0)
    nc.scalar.activation(e, t, AF.Exp, scale=-1.0)
    if split:
        s = pools.tile(shape, FP32, name=f"fm_s_{tag}", tag=f"fm_s_{tag}")
        nc.vector.tensor_add(s, t, e)
        nc.vector.tensor_add(out_ap, x_ap, s)
    else:
        nc.vector.scalar_tensor_tensor(
            out_ap, in0=x_ap, scalar=0.0, in1=e, op0=OP.max, op1=OP.add
        )


def build_kernel(L_=L, S_=S):
    nc = bacc.Bacc(trn_type="TRN2")
    HD = H * D
    q_d = nc.dram_tensor("queries", [L_, HD], FP32, kind="ExternalInput")
    k_d = nc.dram_tensor("keys", [S_, HD], FP32, kind="ExternalInput")
    v_d = nc.dram_tensor("values", [S_, HD], FP32, kind="ExternalInput")
    o_d = nc.dram_tensor("out", [L_, HD], FP32, kind="ExternalOutput")

    n_kc = S_ // 256  # K/V outer iterations (2 chunks of 128 each)
    n_qc = L_ // 256

    with tile.TileContext(nc) as tc:
        with (
            tc.tile_pool(name="consts", bufs=1) as consts,
            tc.tile_pool(name="kdma", bufs=3) as kdma,
            tc.tile_pool(name="vdma", bufs=3) as vdma,
            tc.tile_pool(name="fmk", bufs=2) as fmk,
            tc.tile_pool(name="w2p", bufs=1) as w2p,
            tc.tile_pool(name="qdma", bufs=3) as qdma,
            tc.tile_pool(name="kvpsum", bufs=1, space="PSUM") as kvpsum,
            tc.tile_pool(name="pst", bufs=2, space="PSUM") as pstp,
            tc.tile_pool(name="psum2", bufs=1, space="PSUM") as psum2p,
            tc.tile_pool(name="fmq", bufs=2) as fmq,
            tc.tile_pool(name="qt", bufs=2) as qtp,
            tc.tile_pool(name="zp", bufs=2) as zp,
            tc.tile_pool(name="outp", bufs=3) as outp,
        ):
            ident = consts.tile([P, P], FP32)
            make_identity(nc, ident)

            # ---- Phase 1: KV + Ksum accumulation ----
            # 4 psum tiles, one bank per head PAIR.  One matmul per pair:
            # lhsT = K'[128 s, 128 (2 heads d)], rhs = [V_pair | ones]
            # [128, 129] -> psum [128, 129]: KV_2j at [0:64, 0:64],
            # KV_2j+1 at [64:128, 64:128], Ksums in col 128 (cross blocks
            # are unused garbage).
            kv_ps = [kvpsum.tile([P, 129], FP32, name=f"kv{j}", tag=f"kv{j}") for j in range(4)]

            for cc in range(n_kc):
                r0 = cc * 256
                ktile = kdma.tile([P, 2, HD], FP32, name="ktile", tag="ktile")
                nc.sync.dma_start(
                    ktile,
                    k_d[r0 : r0 + 256, :].rearrange("(two p) f -> p two f", p=P),
                )
                vtile = vdma.tile([P, 2, 4, 129], FP32, name="vtile", tag="vtile")
                nc.vector.memset(vtile[:, :, :, 128:129], 1.0)
                for sub in range(2):
                    nc.sync.dma_start(
                        vtile[:, sub, :, 0:128],
                        v_d[r0 + sub * P : r0 + (sub + 1) * P, :].rearrange(
                            "p (j e) -> p j e", j=4
                        ),
                    )
                kp = fmk.tile([P, 2, H, D], FP32, name="kp", tag="kp")
                _feature_map(
                    nc, fmk, ktile.rearrange("p two (h d) -> p two h d", h=H), kp,
                    [P, 2, H, D], "k", split=True,
                )
                kpf = kp.rearrange("p two h d -> p two (h d)")
                for sub in range(2):
                    for j in range(4):
                        nc.tensor.matmul(
                            kv_ps[j],
                            lhsT=kpf[:, sub, j * P : (j + 1) * P],
                            rhs=vtile[:, sub, j, :],
                            start=(cc == 0 and sub == 0),
                            stop=(cc == n_kc - 1 and sub == 1),
                        )

            # ---- Phase 1.5: build block-diagonal [KV | Ksum] weights ----
            # w2[j] [128, 130]: cols 0:65 = head 2j rows 0:64; cols 65:130 =
            # head 2j+1 rows 64:128; rest zero.
            w2 = [w2p.tile([P, 130], FP32, name=f"w2_{j}", tag=f"w2_{j}") for j in range(4)]
            for j in range(4):
                nc.vector.memset(w2[j], 0.0)
                nc.vector.tensor_copy(w2[j][0:64, 0:64], kv_ps[j][0:64, 0:64])
                nc.vector.tensor_copy(w2[j][0:64, 64:65], kv_ps[j][0:64, 128:129])
                nc.vector.tensor_copy(w2[j][64:128, 65:129], kv_ps[j][64:128, 64:128])
                nc.vector.tensor_copy(w2[j][64:128, 129:130], kv_ps[j][64:128, 128:129])

            # ---- Phase 2: stream Q ----
            for cc in range(n_qc):
                r0 = cc * 256
                qtile = qdma.tile([P, 2, HD], FP32, name="qtile", tag="qtile")
                nc.sync.dma_start(
                    qtile,
                    q_d[r0 : r0 + 256, :].rearrange("(two p) f -> p two f", p=P),
                )
                for sub in range(2):
                    # PE transpose raw Q: [128 l, 128 (2 heads d)] -> [128, 128 l]
                    pst = pstp.tile([P, HD], FP32, name="pst", tag="pst")
                    for g in range(4):
                        nc.tensor.transpose(
                            pst[:, g * P : (g + 1) * P],
                            qtile[:, sub, g * P : (g + 1) * P],
                            ident,
                        )
                    qt = qtp.tile([P, HD], FP32, name="qt", tag="qt")
                    _feature_map(nc, fmq, pst, qt, [P, HD], "q")

                    otile = outp.tile([P, H, D], FP32, name="otile", tag="otile")
                    for g2 in range(2):
                        p2 = psum2p.tile([P, 260], FP32, name=f"p2_{g2}", tag=f"p2_{g2}")
                        for gg in range(2):
                            g = 2 * g2 + gg
                            nc.tensor.matmul(
                                p2[:, gg * 130 : (gg + 1) * 130],
                                lhsT=qt[:, g * P : (g + 1) * P],
                                rhs=w2[g],
                                start=True,
                                stop=True,
                            )
                        p2r = p2.rearrange("p (b c) -> p b c", c=65)
                        zt = zp.tile([P, 4], FP32, name=f"zt{g2}", tag=f"zt{g2}")
                        nc.vector.tensor_scalar_add(zt, p2r[:, :, 64], EPS)
                        zr = zp.tile([P, 4], FP32, name=f"zr{g2}", tag=f"zr{g2}")
                        nc.vector.reciprocal(zr, zt)
                        for b in range(4):
                            nc.vector.tensor_scalar_mul(
                                otile[:, 4 * g2 + b, :],
                                p2r[:, b, 0:64],
                                zr[:, b : b + 1],
                            )
                    nc.sync.dma_start(
                        o_d[r0 + sub * P : r0 + (sub + 1) * P, :],
                        otile.rearrange("p h d -> p (h d)"),
                    )
    nc.compile()
    return nc


@functools.lru_cache(maxsize=None)
def _cached_nc(L_, S_):
    return build_kernel(L_, S_)


def kernel(queries: np.ndarray, keys: np.ndarray, values: np.ndarray) -> np.ndarray:
    n, l_, h, d = queries.shape
    s_ = keys.shape[1]
    nc = _cached_nc(l_, s_)
    in_maps = [
        {
            "queries": np.ascontiguousarray(queries[i].reshape(l_, h * d), np.float32),
            "keys": np.ascontiguousarray(keys[i].reshape(s_, h * d), np.float32),
            "values": np.ascontiguousarray(values[i].reshape(s_, h * d), np.float32),
        }
        for i in range(n)
    ]
    res = run_bass_kernel_spmd(nc, in_maps, core_ids=list(range(n)))
    return np.stack(
        [res.results[i]["out"].reshape(l_, h, d) for i in range(n)]
    ).astype(np.float32)


if __name__ == "__main__":
    # smoke build
    nc = build_kernel()
    print("build ok")
